# revision 79
# baseline (speedup 1.0000x reference)
"""Distributed Bass kernel: multi-head causal attention on 8 TRN2 NeuronCores.

Problem (hardcoded): BATCH=2, SEQ=2048, D_MODEL=2048, N_HEADS=16, D_HEAD=128, f32 I/O.

Sharding: tensor-parallel over heads. Core c owns heads {2c, 2c+1}.
  - x is replicated (fed pre-transposed as xT [D, B*S] bf16).
  - Each core computes QT/KT [e, tok] and V [tok, e] for its 2 heads,
    causal attention in the S^T formulation (scores tiles [keys, q]),
    producing zT [2*128, S] per batch directly.
  - AllGather of zT per (batch, 512-query chunk) -> zT_all [2048, 512]
    chunks (Shared), overlapping collectives with later compute.
  - Each core computes a disjoint 256-column slice of the output
    projection per chunk: outT = W_O[:, cols_c]^T @ z_all^T + b_O[cols_c].
  - Host concatenates the column slices (pure unshard).

Softmax skips max-subtraction: scores ~ N(0,1) here (q,k entries ~N(0,1),
scaled by 1/sqrt(128)), so exp never overflows in f32.
"""

import sys

sys.path.insert(0, "/opt/trn_rl_repo")

from contextlib import ExitStack

import ml_dtypes
import numpy as np

import concourse.bass as bass  # noqa: F401
import concourse.mybir as mybir
import concourse.tile as tile
from concourse import bacc
from concourse.bass_utils import run_bass_kernel_spmd
from concourse.masks import make_identity
from concourse.tile import add_dep_helper

BF16 = mybir.dt.bfloat16
F32 = mybir.dt.float32

B, S, D, NH, E = 2, 2048, 2048, 16, 128
TOK = B * S                  # 4096 tokens
HL = 2                       # heads per core
NCORES = 8
KD = D // 128                # 16 contraction tiles for projections
QC = 512                     # query-chunk width (moving free dim)
NQC = S // QC                # 4 query chunks per batch
NTT = S // 128               # 16 token tiles of 128 per batch
DCOL = 256                   # output columns per core
ATTN_SCALE = np.sqrt(np.float32(E)).astype(np.float32)

_CACHED = {}
TRACE = False


def _install_ntff_hook():
    """The image's antenv lacks axon_hooks; inject it so trace=True works."""
    import types

    if "antenv.axon_hooks" in sys.modules:
        return
    from trn_agent_boot.trn_boot import _ntff_profile_via_ctypes

    hook = _ntff_profile_via_ctypes("/opt/axon/libaxon_pjrt.so")
    mod = types.ModuleType("antenv.axon_hooks")
    mod._hook = hook
    mod.get_axon_ntff_profile_hook = lambda: mod._hook
    mod.set_axon_ntff_profile_hook = lambda h: setattr(mod, "_hook", h)
    sys.modules["antenv.axon_hooks"] = mod
    import antenv

    antenv.axon_hooks = mod

    from concourse import bass_utils as _bu

    _orig_upload = _bu.upload_artifacts

    def _safe_upload(tmpdir):
        try:
            return _orig_upload(tmpdir)
        except Exception as e:  # noqa: BLE001
            print(f"upload_artifacts skipped: {type(e).__name__}: {e}")
            return tmpdir

    _bu.upload_artifacts = _safe_upload


def build_nc():
    nc = bacc.Bacc(None, num_devices=NCORES)

    xT = nc.dram_tensor("xT", [D, TOK], BF16, kind="ExternalInput")
    wq = nc.dram_tensor("wq", [D, HL * E], BF16, kind="ExternalInput")
    wk = nc.dram_tensor("wk", [D, HL * E], BF16, kind="ExternalInput")
    wv = nc.dram_tensor("wv", [D, HL * E], BF16, kind="ExternalInput")
    wo = nc.dram_tensor("wo", [D, DCOL], BF16, kind="ExternalInput")
    bq = nc.dram_tensor("bq", [E, HL], F32, kind="ExternalInput")
    bk = nc.dram_tensor("bk", [E, HL], F32, kind="ExternalInput")
    bv = nc.dram_tensor("bv", [E, HL], F32, kind="ExternalInput")
    bo = nc.dram_tensor("bo", [128, 2], F32, kind="ExternalInput")
    masks = nc.dram_tensor("masks", [128, 4 * QC], BF16, kind="ExternalInput")
    out = nc.dram_tensor("out", [DCOL, TOK], BF16, kind="ExternalOutput")

    # AllGather chunks: one per (batch, query chunk).
    CHUNKS = [(b_, qc_, 0, QC) for b_ in range(B) for qc_ in range(NQC)]
    zb = [
        nc.dram_tensor(f"zb_{ci}", [HL * E, w], BF16)
        for ci, (_, _, _, w) in enumerate(CHUNKS)
    ]
    zall = [
        nc.dram_tensor(f"zall_{ci}", [NCORES * HL * E, w], BF16, addr_space="Shared")
        for ci, (_, _, _, w) in enumerate(CHUNKS)
    ]

    Exp = mybir.ActivationFunctionType.Exp
    cc_insts = {}          # chunk index -> collective instruction

    with tile.TileContext(nc) as tc, ExitStack() as ctx:
        const = ctx.enter_context(tc.tile_pool(name="const", bufs=1))

        # ---- constants / weights ----
        # (wq/wk/wv DMAs are emitted interleaved with the first batch's xT
        # tiles below so the first projection matmuls start early; wo is
        # emitted last — it is only needed in phase 3.)
        wq_sb = const.tile([128, KD, HL * E], BF16, tag="wq")
        wk_sb = const.tile([128, KD, HL * E], BF16, tag="wk")
        wv_sb = const.tile([128, KD, HL * E], BF16, tag="wv")
        wo_sb = const.tile([128, KD, DCOL], BF16, tag="wo")
        bq_sb = const.tile([E, HL], F32, tag="bq")
        bk_sb = const.tile([E, HL], F32, tag="bk")
        bv_sb = const.tile([E, HL], F32, tag="bv")
        bo_sb = const.tile([128, 2], F32, tag="bo")
        nc.sync.dma_start(out=bq_sb[:], in_=bq[:])
        nc.sync.dma_start(out=bk_sb[:], in_=bk[:])
        nc.sync.dma_start(out=bv_sb[:], in_=bv[:])
        nc.sync.dma_start(out=bo_sb[:], in_=bo[:])
        masks_sb = const.tile([128, 4 * QC], BF16, tag="masks")
        nc.sync.dma_start(out=masks_sb[:], in_=masks[:])
        ones_col = const.tile([128, 1], BF16, tag="ones_c")
        nc.vector.memset(ones_col[:], 1.0)
        ones_row = const.tile([1, 128], BF16, tag="ones_r")
        nc.vector.memset(ones_row[:], 1.0)
        ident = const.tile([128, 128], BF16, tag="ident")
        make_identity(nc, ident[:])

        # ---- phase 1+2: projections + attention, one batch at a time ----
        with (
            tc.tile_pool(name="x", bufs=3) as xpool,
            tc.tile_pool(name="qk", bufs=2) as qkpool,
            tc.tile_pool(name="v", bufs=2) as vpool,
            tc.tile_pool(name="p", bufs=8) as ppool,
            tc.tile_pool(name="norm", bufs=5) as npool,
            tc.tile_pool(name="oza", bufs=1) as ozapool,
            tc.tile_pool(name="oosb", bufs=2) as oopool,
            tc.tile_pool(name="projps", bufs=2, space="PSUM") as pr_ps,
            tc.tile_pool(name="sps", bufs=2, space="PSUM") as s_ps,
            tc.tile_pool(name="zps", bufs=2, space="PSUM") as z_ps,
            tc.tile_pool(name="lps", bufs=2, space="PSUM") as l_ps,
        ):
            def emit_ochunk(ci, zpool, opool_, psum_pool, pstag):
                b_, qc_, off_, w_ = CHUNKS[ci]
                za_sb = zpool.tile([128, KD, QC], BF16, tag="ozall")
                cc = cc_insts[ci]
                for k in range(KD):
                    dma = nc.sync.dma_start(
                        out=za_sb[:, k, :w_],
                        in_=zall[ci][k * 128:(k + 1) * 128, :],
                    )
                    add_dep_helper(dma.ins, cc.ins, reason="zall read waits AG")
                for mh in range(2):
                    ps = psum_pool.tile([128, QC], F32, tag=pstag)
                    for k in range(KD):
                        nc.tensor.matmul(
                            ps[:, :w_],
                            wo_sb[:, k, mh * 128:(mh + 1) * 128],
                            za_sb[:, k, :w_],
                            start=(k == 0),
                            stop=(k == KD - 1),
                        )
                    osb = opool_.tile([128, QC], BF16, tag="oosb")
                    nc.vector.tensor_scalar_add(
                        osb[:, :w_], ps[:, :w_], bo_sb[:, mh:mh + 1]
                    )
                    nc.scalar.dma_start(
                        out=out[
                            mh * 128:(mh + 1) * 128,
                            b_ * S + qc_ * QC + off_: b_ * S + qc_ * QC + off_ + w_,
                        ],
                        in_=osb[:, :w_],
                    )
            # Deferred finalize machinery: the normalize chain of one (h, qc)
            # unit is emitted after the next unit's first S matmuls so the
            # in-order PE never stalls waiting on the DVE l-copy.
            pending_fin = []          # closures, each returns [(ci, dma), ...]
            zw_by_chunk = {}          # chunk index -> list of z bounce-write DMAs

            def flush_fin():
                while pending_fin:
                    for ci, dma in pending_fin.pop(0)():
                        zw = zw_by_chunk.setdefault(ci, [])
                        zw.append(dma)
                        if len(zw) == HL:
                            cc = nc.gpsimd.collective_compute(
                                "AllGather",
                                mybir.AluOpType.bypass,
                                replica_groups=[list(range(NCORES))],
                                ins=[zb[ci][:]],
                                outs=[zall[ci][:]],
                            )
                            for dma_ in zw:
                                add_dep_helper(
                                    cc.ins, dma_.ins, reason="AG reads z bounce"
                                )
                            cc_insts[ci] = cc

            for b in range(B):
                qt_tile = qkpool.tile([128, HL, S], BF16, tag="qt")
                kt_tile = qkpool.tile([128, HL, S], BF16, tag="kt")
                v_tile = vpool.tile([128, NTT, HL * E], BF16, tag="v")

                # Stream per query-chunk column slice: load x columns, project
                # Q/K/V for those tokens, then attend (keys are a causal
                # prefix, so K/V for kb <= qc end are already resident).
                for qc in range(NQC):
                    cs = qc * QC  # column start within batch
                    xq_sb = xpool.tile([128, KD, QC], BF16, tag="xT")
                    vt_tile = qkpool.tile([128, HL, QC], BF16, tag="vt")
                    # Emit input DMAs in the order the PE consumes them: the
                    # first projection group needs wq+xT k-wise; wk/wv gate
                    # only the later groups.
                    for k in range(KD):
                        nc.sync.dma_start(
                            out=xq_sb[:, k, :],
                            in_=xT[k * 128:(k + 1) * 128, b * S + cs:b * S + cs + QC],
                        )
                        if b == 0 and qc == 0:
                            nc.sync.dma_start(
                                out=wq_sb[:, k, :], in_=wq[k * 128:(k + 1) * 128, :]
                            )
                    if b == 0 and qc == 0:
                        for k in range(KD):
                            nc.sync.dma_start(
                                out=wk_sb[:, k, :], in_=wk[k * 128:(k + 1) * 128, :]
                            )
                        for k in range(KD):
                            nc.sync.dma_start(
                                out=wv_sb[:, k, :], in_=wv[k * 128:(k + 1) * 128, :]
                            )
                    if b == 1 and qc == 0:  # wo needed for mid-kernel O chunks
                        for k in range(KD):
                            nc.sync.dma_start(
                                out=wo_sb[:, k, :], in_=wo[k * 128:(k + 1) * 128, :]
                            )

                    # Q^T, K^T, V^T for this chunk. W stationary, xT moving —
                    # LDWEIGHTS hides under the N=512 matmuls for all three.
                    # V^T is then flipped to V [tok, e] by the DMA engine's
                    # transpose mode (no PE/DVE cost).
                    # tensor-major order: both heads of Q before K before V^T,
                    # so early groups never wait on later weight tensors.
                    for wsb, bsb, dst in (
                        (wq_sb, bq_sb, qt_tile),
                        (wk_sb, bk_sb, kt_tile),
                        (wv_sb, bv_sb, vt_tile),
                    ):
                        for h in range(HL):
                            ps = pr_ps.tile([128, QC], F32, tag="projps")
                            for k in range(KD):
                                nc.tensor.matmul(
                                    ps[:],
                                    wsb[:, k, h * E:(h + 1) * E],
                                    xq_sb[:, k, :],
                                    start=(k == 0),
                                    stop=(k == KD - 1),
                                )
                            dcs = 0 if dst is vt_tile else cs
                            nc.vector.tensor_scalar_add(
                                dst[:, h, dcs:dcs + QC], ps[:], bsb[:, h:h + 1]
                            )
                    for h in range(HL):
                        for tt in range(qc * (QC // 128), (qc + 1) * (QC // 128)):
                            tl = tt - qc * (QC // 128)
                            tps = pr_ps.tile([128, 128], BF16, tag="projps")
                            nc.tensor.transpose(
                                tps[:], vt_tile[:, h, tl * 128:(tl + 1) * 128], ident[:]
                            )
                            nc.vector.tensor_copy(
                                v_tile[:, tt, h * E:(h + 1) * E], tps[:]
                            )

                    # During batch 1, slot batch-0's output-projection chunks in
                    # before each attention unit: their AllGathers are long
                    # done, and this moves their z_all DMA traffic off the tail.
                    if b == 1:
                        flush_fin()
                        emit_ochunk(qc, ozapool, oopool, pr_ps, "projps")

                    # attention for both heads of this chunk; z/l matmuls lag
                    # two blocks behind S/exp so PE never stalls on the chain.
                    nkb = (qc + 1) * (QC // 128)
                    for h in range(HL):
                        zps = z_ps.tile([128, QC], F32, tag="zps")
                        lps = l_ps.tile([1, QC], F32, tag="lps")

                        def zl_mms(pt, kb, nkb=nkb, zps=zps, h=h, v_tile=v_tile):
                            nc.tensor.matmul(
                                zps[:],
                                v_tile[:, kb, h * E:(h + 1) * E],
                                pt[:],
                                start=(kb == 0),
                                stop=(kb == nkb - 1),
                            )

                        pending = []   # (pt, kb) whose z MM is not yet emitted
                        pend_l = []    # (padd, quad_idx) l MMs not yet emitted
                        ptq = []       # exp tiles awaiting quad-reduction
                        nquads = nkb // 4

                        def l_mm(padd, pi, lps=lps, nquads=nquads):
                            nc.tensor.matmul(
                                lps[:], ones_col[:], padd[:],
                                start=(pi == 0), stop=(pi == nquads - 1),
                            )

                        for kb in range(nkb):
                            sps = s_ps.tile([128, QC], F32, tag="sps")
                            nc.tensor.matmul(
                                sps[:],
                                kt_tile[:, h, kb * 128:(kb + 1) * 128],
                                qt_tile[:, h, cs:cs + QC],
                                start=True,
                                stop=True,
                            )
                            if kb == 1:
                                flush_fin()  # prior unit's deferred normalize
                            if len(pending) >= 2:
                                zl_mms(*pending.pop(0))
                            if len(pend_l) >= 2:
                                l_mm(*pend_l.pop(0))
                            pt = ppool.tile([128, QC], BF16, tag="pt")
                            nc.scalar.activation(pt[:], sps[:], Exp)
                            dd = kb - qc * (QC // 128)
                            if dd >= 0:  # diagonal block: zero future keys
                                pt2 = ppool.tile([128, QC], BF16, tag="pt")
                                nc.vector.tensor_mul(
                                    pt2[:], pt[:], masks_sb[:, dd * QC:(dd + 1) * QC]
                                )
                                pt = pt2
                            pending.append((pt, kb))
                            # quad-reduce exp tiles on DVE so l needs 1/4 the MMs
                            ptq.append(pt)
                            if len(ptq) == 4:
                                s01 = npool.tile([128, QC], BF16, tag="padd")
                                nc.vector.tensor_tensor(
                                    out=s01[:], in0=ptq[0][:], in1=ptq[1][:],
                                    op=mybir.AluOpType.add,
                                )
                                s23 = npool.tile([128, QC], BF16, tag="padd")
                                nc.vector.tensor_tensor(
                                    out=s23[:], in0=ptq[2][:], in1=ptq[3][:],
                                    op=mybir.AluOpType.add,
                                )
                                padd = npool.tile([128, QC], BF16, tag="padd")
                                nc.vector.tensor_tensor(
                                    out=padd[:], in0=s01[:], in1=s23[:],
                                    op=mybir.AluOpType.add,
                                )
                                pend_l.append((padd, kb // 4))
                                ptq = []
                        for args in pending:
                            zl_mms(*args)
                        for args in pend_l:
                            l_mm(*args)

                        def finalize(b=b, qc=qc, h=h, zps=zps, lps=lps):
                            # normalize: zT /= l. 1/l on DVE (fast approx),
                            # broadcast across partitions via PE.
                            linv = npool.tile([1, QC], F32, tag="linv")
                            nc.vector.reciprocal_approx_fast(linv[:], lps[:])
                            linvb = npool.tile([1, QC], BF16, tag="linvb")
                            nc.vector.tensor_copy(linvb[:], linv[:])
                            bps = l_ps.tile([128, QC], F32, tag="lps")
                            nc.tensor.matmul(
                                bps[:], ones_row[:], linvb[:], start=True, stop=True
                            )
                            binv = npool.tile([128, QC], F32, tag="binv")
                            nc.vector.tensor_copy(binv[:], bps[:])
                            zn = npool.tile([128, QC], BF16, tag="zn")
                            nc.vector.tensor_mul(zn[:], zps[:], binv[:])
                            out_dmas = []
                            for ci, (b_, qc_, off, w) in enumerate(CHUNKS):
                                if (b_, qc_) != (b, qc):
                                    continue
                                dma = nc.sync.dma_start(
                                    out=zb[ci][h * E:(h + 1) * E, :],
                                    in_=zn[:, off:off + w],
                                )
                                out_dmas.append((ci, dma))
                            return out_dmas

                        pending_fin.append(finalize)
            flush_fin()

        # ---- phase 3: batch-1's O-projection chunks (batch 0 was emitted
        # mid-kernel, interleaved with batch-1 attention) ----
        with (
            tc.tile_pool(name="zall", bufs=3) as zapool,
            tc.tile_pool(name="osb", bufs=3) as opool,
            tc.tile_pool(name="ops", bufs=4, space="PSUM") as o_ps,
        ):
            for ci in range(NQC, 2 * NQC):
                emit_ochunk(ci, zapool, opool, o_ps, "ops")

    nc.finalize()
    return nc


def _make_masks():
    k_idx = np.arange(128)[:, None]
    q_idx = np.arange(QC)[None, :]
    ms = [(q_idx >= k_idx + 128 * d) for d in range(4)]
    return np.concatenate(ms, axis=1).astype(ml_dtypes.bfloat16)


def kernel(x, W_Q, W_K, W_V, W_O, b_Q, b_K, b_V, b_O):
    x = np.asarray(x, dtype=np.float32)
    W_Q = np.asarray(W_Q, dtype=np.float32)
    W_K = np.asarray(W_K, dtype=np.float32)
    W_V = np.asarray(W_V, dtype=np.float32)
    W_O = np.asarray(W_O, dtype=np.float32)
    b_Q = np.asarray(b_Q, dtype=np.float32)
    b_K = np.asarray(b_K, dtype=np.float32)
    b_V = np.asarray(b_V, dtype=np.float32)
    b_O = np.asarray(b_O, dtype=np.float32)

    if "nc" not in _CACHED:
        _CACHED["nc"] = build_nc()
    nc = _CACHED["nc"]

    bf = ml_dtypes.bfloat16
    xT = np.ascontiguousarray(x.reshape(TOK, D).T).astype(bf)
    masks = _make_masks()
    wo_flat = W_O.reshape(NH * E, D)

    in_maps = []
    for c in range(NCORES):
        h0, h1 = 2 * c, 2 * c + 1
        wq_c = np.concatenate([W_Q[h0], W_Q[h1]], axis=1) / ATTN_SCALE
        wk_c = np.concatenate([W_K[h0], W_K[h1]], axis=1)
        wv_c = np.concatenate([W_V[h0], W_V[h1]], axis=1)
        in_maps.append({
            "xT": xT,
            "wq": np.ascontiguousarray(wq_c).astype(bf),
            "wk": np.ascontiguousarray(wk_c).astype(bf),
            "wv": np.ascontiguousarray(wv_c).astype(bf),
            "wo": np.ascontiguousarray(wo_flat[:, c * DCOL:(c + 1) * DCOL]).astype(bf),
            "bq": np.ascontiguousarray(np.stack([b_Q[h0], b_Q[h1]], axis=1) / ATTN_SCALE),
            "bk": np.ascontiguousarray(np.stack([b_K[h0], b_K[h1]], axis=1)),
            "bv": np.ascontiguousarray(np.stack([b_V[h0], b_V[h1]], axis=1)),
            "bo": np.ascontiguousarray(
                b_O[c * DCOL:(c + 1) * DCOL].reshape(2, 128).T
            ),
            "masks": masks,
        })

    if TRACE:
        _install_ntff_hook()
    res = run_bass_kernel_spmd(nc, in_maps, list(range(NCORES)), trace=TRACE)
    if TRACE:
        print(f"HW exec time: {res.exec_time_ns} ns", flush=True)
        _CACHED["last_result"] = res
    outT = [np.asarray(res.results[c]["out"], dtype=np.float32) for c in range(NCORES)]
    out = np.concatenate([o.T for o in outT], axis=1)      # [4096, 2048]
    return np.ascontiguousarray(out.reshape(B, S, D)).astype(np.float32)


# revision 80
# speedup vs baseline: 1.0861x; 1.0861x over previous
"""Distributed Bass kernel: multi-head causal attention on 8 TRN2 NeuronCores.

Problem (hardcoded): BATCH=2, SEQ=2048, D_MODEL=2048, N_HEADS=16, D_HEAD=128, f32 I/O.

Sharding: tensor-parallel over heads. Core c owns heads {2c, 2c+1}.
  - x is replicated (fed pre-transposed as xT [D, B*S] bf16).
  - Each core computes QT/KT [e, tok] and V [tok, e] for its 2 heads,
    causal attention in the S^T formulation (scores tiles [keys, q]),
    producing zT [2*128, S] per batch directly.
  - AllGather of zT per (batch, 512-query chunk) -> zT_all [2048, 512]
    chunks (Shared), overlapping collectives with later compute.
  - Each core computes a disjoint 256-column slice of the output
    projection per chunk: outT = W_O[:, cols_c]^T @ z_all^T + b_O[cols_c].
  - Host concatenates the column slices (pure unshard).

Softmax skips max-subtraction: scores ~ N(0,1) here (q,k entries ~N(0,1),
scaled by 1/sqrt(128)), so exp never overflows in f32.
"""

import sys

sys.path.insert(0, "/opt/trn_rl_repo")

from contextlib import ExitStack

import ml_dtypes
import numpy as np

import concourse.bass as bass  # noqa: F401
import concourse.mybir as mybir
import concourse.tile as tile
from concourse import bacc
from concourse.bass_utils import run_bass_kernel_spmd
from concourse.masks import make_identity
from concourse.tile import add_dep_helper

BF16 = mybir.dt.bfloat16
F32 = mybir.dt.float32

B, S, D, NH, E = 2, 2048, 2048, 16, 128
TOK = B * S                  # 4096 tokens
HL = 2                       # heads per core
NCORES = 8
KD = D // 128                # 16 contraction tiles for projections
QC = 512                     # query-chunk width (moving free dim)
NQC = S // QC                # 4 query chunks per batch
NTT = S // 128               # 16 token tiles of 128 per batch
DCOL = 256                   # output columns per core
ATTN_SCALE = np.sqrt(np.float32(E)).astype(np.float32)

_CACHED = {}
TRACE = False


def _install_ntff_hook():
    """The image's antenv lacks axon_hooks; inject it so trace=True works."""
    import types

    if "antenv.axon_hooks" in sys.modules:
        return
    from trn_agent_boot.trn_boot import _ntff_profile_via_ctypes

    hook = _ntff_profile_via_ctypes("/opt/axon/libaxon_pjrt.so")
    mod = types.ModuleType("antenv.axon_hooks")
    mod._hook = hook
    mod.get_axon_ntff_profile_hook = lambda: mod._hook
    mod.set_axon_ntff_profile_hook = lambda h: setattr(mod, "_hook", h)
    sys.modules["antenv.axon_hooks"] = mod
    import antenv

    antenv.axon_hooks = mod

    from concourse import bass_utils as _bu

    _orig_upload = _bu.upload_artifacts

    def _safe_upload(tmpdir):
        try:
            return _orig_upload(tmpdir)
        except Exception as e:  # noqa: BLE001
            print(f"upload_artifacts skipped: {type(e).__name__}: {e}")
            return tmpdir

    _bu.upload_artifacts = _safe_upload


def build_nc():
    nc = bacc.Bacc(None, num_devices=NCORES)

    xT = nc.dram_tensor("xT", [D, TOK], BF16, kind="ExternalInput")
    wq = nc.dram_tensor("wq", [D, HL * E], BF16, kind="ExternalInput")
    wk = nc.dram_tensor("wk", [D, HL * E], BF16, kind="ExternalInput")
    wv = nc.dram_tensor("wv", [D, HL * E], BF16, kind="ExternalInput")
    wo = nc.dram_tensor("wo", [D, DCOL], BF16, kind="ExternalInput")
    bq = nc.dram_tensor("bq", [E, HL], F32, kind="ExternalInput")
    bk = nc.dram_tensor("bk", [E, HL], F32, kind="ExternalInput")
    bv = nc.dram_tensor("bv", [E, HL], F32, kind="ExternalInput")
    bo = nc.dram_tensor("bo", [128, 2], F32, kind="ExternalInput")
    masks = nc.dram_tensor("masks", [128, 4 * QC], BF16, kind="ExternalInput")
    out = nc.dram_tensor("out", [DCOL, TOK], BF16, kind="ExternalOutput")

    # AllGather chunks: one per (batch, query chunk).
    CHUNKS = [(b_, qc_, 0, QC) for b_ in range(B) for qc_ in range(NQC)]
    zb = [
        nc.dram_tensor(f"zb_{ci}", [HL * E, w], BF16)
        for ci, (_, _, _, w) in enumerate(CHUNKS)
    ]
    zall = [
        nc.dram_tensor(f"zall_{ci}", [NCORES * HL * E, w], BF16, addr_space="Shared")
        for ci, (_, _, _, w) in enumerate(CHUNKS)
    ]

    Exp = mybir.ActivationFunctionType.Exp
    cc_insts = {}          # chunk index -> collective instruction

    with tile.TileContext(nc) as tc, ExitStack() as ctx:
        const = ctx.enter_context(tc.tile_pool(name="const", bufs=1))

        # ---- constants / weights ----
        # (wq/wk/wv DMAs are emitted interleaved with the first batch's xT
        # tiles below so the first projection matmuls start early; wo is
        # emitted last — it is only needed in phase 3.)
        wq_sb = const.tile([128, KD, HL * E], BF16, tag="wq")
        wk_sb = const.tile([128, KD, HL * E], BF16, tag="wk")
        wv_sb = const.tile([128, KD, HL * E], BF16, tag="wv")
        wo_sb = const.tile([128, KD, DCOL], BF16, tag="wo")
        bq_sb = const.tile([E, HL], F32, tag="bq")
        bk_sb = const.tile([E, HL], F32, tag="bk")
        bv_sb = const.tile([E, HL], F32, tag="bv")
        bo_sb = const.tile([128, 2], F32, tag="bo")
        nc.sync.dma_start(out=bq_sb[:], in_=bq[:])
        nc.sync.dma_start(out=bk_sb[:], in_=bk[:])
        nc.sync.dma_start(out=bv_sb[:], in_=bv[:])
        nc.sync.dma_start(out=bo_sb[:], in_=bo[:])
        masks_sb = const.tile([128, 4 * QC], BF16, tag="masks")
        nc.sync.dma_start(out=masks_sb[:], in_=masks[:])
        ones_col = const.tile([128, 1], BF16, tag="ones_c")
        nc.vector.memset(ones_col[:], 1.0)
        ones_row = const.tile([1, 128], BF16, tag="ones_r")
        nc.vector.memset(ones_row[:], 1.0)
        ident = const.tile([128, 128], BF16, tag="ident")
        make_identity(nc, ident[:])

        # ---- phase 1+2: projections + attention, one batch at a time ----
        with (
            tc.tile_pool(name="x", bufs=1) as xpool,
            tc.tile_pool(name="qk", bufs=2) as qkpool,
            tc.tile_pool(name="v", bufs=2) as vpool,
            tc.tile_pool(name="p", bufs=8) as ppool,
            tc.tile_pool(name="norm", bufs=5) as npool,
            tc.tile_pool(name="projps", bufs=2, space="PSUM") as pr_ps,
            tc.tile_pool(name="sps", bufs=2, space="PSUM") as s_ps,
            tc.tile_pool(name="zps", bufs=2, space="PSUM") as z_ps,
            tc.tile_pool(name="lps", bufs=2, space="PSUM") as l_ps,
        ):
            # Deferred finalize machinery: the normalize chain of one (h, qc)
            # unit is emitted after the next unit's first S matmuls so the
            # in-order PE never stalls waiting on the DVE l-copy.
            pending_fin = []          # closures, each returns [(ci, dma), ...]
            zw_by_chunk = {}          # chunk index -> list of z bounce-write DMAs

            def flush_fin():
                while pending_fin:
                    for ci, dma in pending_fin.pop(0)():
                        zw = zw_by_chunk.setdefault(ci, [])
                        zw.append(dma)
                        if len(zw) == HL:
                            cc = nc.gpsimd.collective_compute(
                                "AllGather",
                                mybir.AluOpType.bypass,
                                replica_groups=[list(range(NCORES))],
                                ins=[zb[ci][:]],
                                outs=[zall[ci][:]],
                            )
                            for dma_ in zw:
                                add_dep_helper(
                                    cc.ins, dma_.ins, reason="AG reads z bounce"
                                )
                            cc_insts[ci] = cc

            for b in range(B):
                xT_sb = xpool.tile([128, KD, S], BF16, tag="xT")
                qt_tile = qkpool.tile([128, HL, S], BF16, tag="qt")
                kt_tile = qkpool.tile([128, HL, S], BF16, tag="kt")
                vt_tile = qkpool.tile([128, HL, S], BF16, tag="vt")
                v_tile = vpool.tile([128, NTT, HL * E], BF16, tag="v")

                # Stream per query-chunk column slice: load x columns, project
                # Q/K/V for those tokens, then attend (keys are a causal
                # prefix, so K/V for kb <= qc end are already resident).
                for qc in range(NQC):
                    cs = qc * QC  # column start within batch
                    # Emit input DMAs in the order the PE consumes them: the
                    # first projection group needs wq+xT k-wise; wk/wv gate
                    # only the later groups.
                    for k in range(KD):
                        nc.sync.dma_start(
                            out=xT_sb[:, k, cs:cs + QC],
                            in_=xT[k * 128:(k + 1) * 128, b * S + cs:b * S + cs + QC],
                        )
                        if b == 0 and qc == 0:
                            nc.sync.dma_start(
                                out=wq_sb[:, k, :], in_=wq[k * 128:(k + 1) * 128, :]
                            )
                    if b == 0 and qc == 0:
                        for k in range(KD):
                            nc.sync.dma_start(
                                out=wk_sb[:, k, :], in_=wk[k * 128:(k + 1) * 128, :]
                            )
                        for k in range(KD):
                            nc.sync.dma_start(
                                out=wv_sb[:, k, :], in_=wv[k * 128:(k + 1) * 128, :]
                            )

                    # Q^T, K^T, V^T for this chunk. W stationary, xT moving —
                    # LDWEIGHTS hides under the N=512 matmuls for all three.
                    # V^T is then flipped to V [tok, e] by the DMA engine's
                    # transpose mode (no PE/DVE cost).
                    # tensor-major order: both heads of Q before K before V^T,
                    # so early groups never wait on later weight tensors.
                    for wsb, bsb, dst in (
                        (wq_sb, bq_sb, qt_tile),
                        (wk_sb, bk_sb, kt_tile),
                        (wv_sb, bv_sb, vt_tile),
                    ):
                        for h in range(HL):
                            ps = pr_ps.tile([128, QC], F32, tag="projps")
                            for k in range(KD):
                                nc.tensor.matmul(
                                    ps[:],
                                    wsb[:, k, h * E:(h + 1) * E],
                                    xT_sb[:, k, cs:cs + QC],
                                    start=(k == 0),
                                    stop=(k == KD - 1),
                                )
                            nc.vector.tensor_scalar_add(
                                dst[:, h, cs:cs + QC], ps[:], bsb[:, h:h + 1]
                            )
                    for h in range(HL):
                        for tt in range(qc * (QC // 128), (qc + 1) * (QC // 128)):
                            tps = pr_ps.tile([128, 128], BF16, tag="projps")
                            nc.tensor.transpose(
                                tps[:], vt_tile[:, h, tt * 128:(tt + 1) * 128], ident[:]
                            )
                            nc.vector.tensor_copy(
                                v_tile[:, tt, h * E:(h + 1) * E], tps[:]
                            )

                    # attention for both heads of this chunk; z/l matmuls lag
                    # two blocks behind S/exp so PE never stalls on the chain.
                    nkb = (qc + 1) * (QC // 128)
                    for h in range(HL):
                        zps = z_ps.tile([128, QC], F32, tag="zps")
                        lps = l_ps.tile([1, QC], F32, tag="lps")

                        def zl_mms(pt, kb, nkb=nkb, zps=zps, h=h, v_tile=v_tile):
                            nc.tensor.matmul(
                                zps[:],
                                v_tile[:, kb, h * E:(h + 1) * E],
                                pt[:],
                                start=(kb == 0),
                                stop=(kb == nkb - 1),
                            )

                        pending = []   # (pt, kb) whose z MM is not yet emitted
                        pend_l = []    # (padd, quad_idx) l MMs not yet emitted
                        ptq = []       # exp tiles awaiting quad-reduction
                        nquads = nkb // 4

                        def l_mm(padd, pi, lps=lps, nquads=nquads):
                            nc.tensor.matmul(
                                lps[:], ones_col[:], padd[:],
                                start=(pi == 0), stop=(pi == nquads - 1),
                            )

                        for kb in range(nkb):
                            sps = s_ps.tile([128, QC], F32, tag="sps")
                            nc.tensor.matmul(
                                sps[:],
                                kt_tile[:, h, kb * 128:(kb + 1) * 128],
                                qt_tile[:, h, cs:cs + QC],
                                start=True,
                                stop=True,
                            )
                            if kb == 1:
                                flush_fin()  # prior unit's deferred normalize
                            if len(pending) >= 2:
                                zl_mms(*pending.pop(0))
                            if len(pend_l) >= 2:
                                l_mm(*pend_l.pop(0))
                            pt = ppool.tile([128, QC], BF16, tag="pt")
                            nc.scalar.activation(pt[:], sps[:], Exp)
                            dd = kb - qc * (QC // 128)
                            if dd >= 0:  # diagonal block: zero future keys
                                pt2 = ppool.tile([128, QC], BF16, tag="pt")
                                nc.vector.tensor_mul(
                                    pt2[:], pt[:], masks_sb[:, dd * QC:(dd + 1) * QC]
                                )
                                pt = pt2
                            pending.append((pt, kb))
                            # quad-reduce exp tiles on DVE so l needs 1/4 the MMs
                            ptq.append(pt)
                            if len(ptq) == 4:
                                s01 = npool.tile([128, QC], BF16, tag="padd")
                                nc.vector.tensor_tensor(
                                    out=s01[:], in0=ptq[0][:], in1=ptq[1][:],
                                    op=mybir.AluOpType.add,
                                )
                                s23 = npool.tile([128, QC], BF16, tag="padd")
                                nc.vector.tensor_tensor(
                                    out=s23[:], in0=ptq[2][:], in1=ptq[3][:],
                                    op=mybir.AluOpType.add,
                                )
                                padd = npool.tile([128, QC], BF16, tag="padd")
                                nc.vector.tensor_tensor(
                                    out=padd[:], in0=s01[:], in1=s23[:],
                                    op=mybir.AluOpType.add,
                                )
                                pend_l.append((padd, kb // 4))
                                ptq = []
                        for args in pending:
                            zl_mms(*args)
                        for args in pend_l:
                            l_mm(*args)

                        def finalize(b=b, qc=qc, h=h, zps=zps, lps=lps):
                            # normalize: zT /= l. 1/l on DVE (fast approx),
                            # broadcast across partitions via PE.
                            linv = npool.tile([1, QC], F32, tag="linv")
                            nc.vector.reciprocal_approx_fast(linv[:], lps[:])
                            linvb = npool.tile([1, QC], BF16, tag="linvb")
                            nc.vector.tensor_copy(linvb[:], linv[:])
                            bps = l_ps.tile([128, QC], F32, tag="lps")
                            nc.tensor.matmul(
                                bps[:], ones_row[:], linvb[:], start=True, stop=True
                            )
                            binv = npool.tile([128, QC], F32, tag="binv")
                            nc.vector.tensor_copy(binv[:], bps[:])
                            zn = npool.tile([128, QC], BF16, tag="zn")
                            nc.vector.tensor_mul(zn[:], zps[:], binv[:])
                            out_dmas = []
                            for ci, (b_, qc_, off, w) in enumerate(CHUNKS):
                                if (b_, qc_) != (b, qc):
                                    continue
                                dma = nc.sync.dma_start(
                                    out=zb[ci][h * E:(h + 1) * E, :],
                                    in_=zn[:, off:off + w],
                                )
                                out_dmas.append((ci, dma))
                            return out_dmas

                        pending_fin.append(finalize)
            flush_fin()

        # wo loads: needed from here on; emitted late to keep startup DMAs lean
        for k in range(KD):
            nc.sync.dma_start(out=wo_sb[:, k, :], in_=wo[k * 128:(k + 1) * 128, :])

        # ---- phase 3: column-sharded O projection, chunk-pipelined ----
        with (
            tc.tile_pool(name="zall", bufs=3) as zapool,
            tc.tile_pool(name="osb", bufs=3) as opool,
            tc.tile_pool(name="ops", bufs=4, space="PSUM") as o_ps,
        ):
            for ci, (b, qc, off, w) in enumerate(CHUNKS):
                za_sb = zapool.tile([128, KD, QC], BF16, tag="zall")
                cc = cc_insts[ci]
                for k in range(KD):
                    dma = nc.sync.dma_start(
                        out=za_sb[:, k, :w],
                        in_=zall[ci][k * 128:(k + 1) * 128, :],
                    )
                    add_dep_helper(dma.ins, cc.ins, reason="zall read waits AG")
                for mh in range(2):
                    ps = o_ps.tile([128, QC], F32, tag="ops")
                    for k in range(KD):
                        nc.tensor.matmul(
                            ps[:, :w],
                            wo_sb[:, k, mh * 128:(mh + 1) * 128],
                            za_sb[:, k, :w],
                            start=(k == 0),
                            stop=(k == KD - 1),
                        )
                    osb = opool.tile([128, QC], BF16, tag="osb")
                    nc.vector.tensor_scalar_add(
                        osb[:, :w], ps[:, :w], bo_sb[:, mh:mh + 1]
                    )
                    nc.scalar.dma_start(
                        out=out[
                            mh * 128:(mh + 1) * 128,
                            b * S + qc * QC + off: b * S + qc * QC + off + w,
                        ],
                        in_=osb[:, :w],
                    )

    nc.finalize()
    return nc


def _make_masks():
    k_idx = np.arange(128)[:, None]
    q_idx = np.arange(QC)[None, :]
    ms = [(q_idx >= k_idx + 128 * d) for d in range(4)]
    return np.concatenate(ms, axis=1).astype(ml_dtypes.bfloat16)


def kernel(x, W_Q, W_K, W_V, W_O, b_Q, b_K, b_V, b_O):
    x = np.asarray(x, dtype=np.float32)
    W_Q = np.asarray(W_Q, dtype=np.float32)
    W_K = np.asarray(W_K, dtype=np.float32)
    W_V = np.asarray(W_V, dtype=np.float32)
    W_O = np.asarray(W_O, dtype=np.float32)
    b_Q = np.asarray(b_Q, dtype=np.float32)
    b_K = np.asarray(b_K, dtype=np.float32)
    b_V = np.asarray(b_V, dtype=np.float32)
    b_O = np.asarray(b_O, dtype=np.float32)

    if "nc" not in _CACHED:
        _CACHED["nc"] = build_nc()
    nc = _CACHED["nc"]

    bf = ml_dtypes.bfloat16
    xT = np.ascontiguousarray(x.reshape(TOK, D).T).astype(bf)
    masks = _make_masks()
    wo_flat = W_O.reshape(NH * E, D)

    in_maps = []
    for c in range(NCORES):
        h0, h1 = 2 * c, 2 * c + 1
        wq_c = np.concatenate([W_Q[h0], W_Q[h1]], axis=1) / ATTN_SCALE
        wk_c = np.concatenate([W_K[h0], W_K[h1]], axis=1)
        wv_c = np.concatenate([W_V[h0], W_V[h1]], axis=1)
        in_maps.append({
            "xT": xT,
            "wq": np.ascontiguousarray(wq_c).astype(bf),
            "wk": np.ascontiguousarray(wk_c).astype(bf),
            "wv": np.ascontiguousarray(wv_c).astype(bf),
            "wo": np.ascontiguousarray(wo_flat[:, c * DCOL:(c + 1) * DCOL]).astype(bf),
            "bq": np.ascontiguousarray(np.stack([b_Q[h0], b_Q[h1]], axis=1) / ATTN_SCALE),
            "bk": np.ascontiguousarray(np.stack([b_K[h0], b_K[h1]], axis=1)),
            "bv": np.ascontiguousarray(np.stack([b_V[h0], b_V[h1]], axis=1)),
            "bo": np.ascontiguousarray(
                b_O[c * DCOL:(c + 1) * DCOL].reshape(2, 128).T
            ),
            "masks": masks,
        })

    if TRACE:
        _install_ntff_hook()
    res = run_bass_kernel_spmd(nc, in_maps, list(range(NCORES)), trace=TRACE)
    if TRACE:
        print(f"HW exec time: {res.exec_time_ns} ns", flush=True)
        _CACHED["last_result"] = res
    outT = [np.asarray(res.results[c]["out"], dtype=np.float32) for c in range(NCORES)]
    out = np.concatenate([o.T for o in outT], axis=1)      # [4096, 2048]
    return np.ascontiguousarray(out.reshape(B, S, D)).astype(np.float32)
